# revision 26
# baseline (speedup 1.0000x reference)
"""Trainium2 Bass kernel for a dense transformer block (B=2, T=2048, C=1024, H=16).

Sharding: DP2 (batch -> core groups {0-3},{4-7}) x TP4 within a group:
  - attention: Megatron head-parallel (4 heads/core), row-parallel out-proj,
    pipelined ReduceScatter(add) over the group (one RS per 512-row block,
    issued immediately after that query-chunk's attention).
  - MLP: sequence-parallel (each core computes its 512 rows with the FULL
    fc / proj weights). No other collective.

Row ownership: core at group position p owns rows {512j+128p .. 512j+128p+128}
for j in 0..3 (one 128-row strip per pipelined ReduceScatter).

Device layout notes:
  - Activations feeding matmuls are kept transposed [features, tokens]
    ("^T layout") so every matmul contracts over the partition dim.
  - LN affine params are folded into the following weights on the host;
    q-scale (1/sqrt(D)) folded into W_q/b_q; v-bias folded into b_proj.
  - Softmax: scores^T[k,q] tiles; exp on ScalarE (no max subtraction:
    scores are ~N(0,1), safe); denominator via ones-column appended to V
    (row 64 of the PV matmul output); normalization applied to y^T with a
    DRAM-bounced partition-broadcast of 1/denom.
  - Matmul operands are fp16 (full PE rate, fast weight load); all
    accumulation, softmax statistics, residuals and LN are fp32.
  - x arrives fp16 (host-cast) to cut first-tile DMA latency; the fp32
    residual base (xres) is a separate input.
  - Scheduling: proj+RS issued per chunk right after its attention; MLP
    weights streamed once (strip-3 fc wave reuses the 2 resident groups);
    single wmp stream covers all 4 strips.
"""

import os
import sys

import numpy as np

for _p in ("/opt/trn_rl_repo", "/root/.axon_site/_ro/trn_rl_repo"):
    if os.path.isdir(_p) and _p not in sys.path:
        sys.path.insert(0, _p)

import concourse.bass as bass
import concourse.tile as tile
from concourse import bacc, mybir
from concourse.bass_utils import run_bass_kernel_spmd

B, T, C, H = 2, 2048, 1024, 16
D = C // H  # 64
EPS = 1e-5
N_CORES = 8
TP = 4            # tensor-parallel group size
HPC = 4           # heads per core
ROWS = T // TP    # 512 token rows owned per core
F32 = mybir.dt.float32
F16 = mybir.dt.float16  # matmul operand dtype

TT = T // 128     # 16 token tiles
CB = C // 128     # 8 channel blocks
QC = T // 512     # 4 query chunks / row blocks
RG = [[0, 1, 2, 3], [4, 5, 6, 7]]

GELU_NAME = "Gelu_apprx_tanh"  # sim_check overrides (sim lacks Gelu)


def _bc(ap, p):
    """Broadcast a DRAM AP across p partitions (prepend stride-0 dim)."""
    return bass.AP(tensor=ap.tensor, offset=ap.offset, ap=[[0, p], *ap.ap])


def build_program():
    nc = bacc.Bacc(
        "TRN2", target_bir_lowering=False, debug=False, num_devices=N_CORES
    )

    # ---- I/O ----
    x_d = nc.dram_tensor("x", [T, C], F16, kind="ExternalInput").ap()
    wqk_d = nc.dram_tensor("wqk", [C, 512], F16, kind="ExternalInput").ap()
    bqk_d = nc.dram_tensor("bqk", [512], F32, kind="ExternalInput").ap()
    wv_d = nc.dram_tensor("wv", [C, 256], F16, kind="ExternalInput").ap()
    wproj_d = nc.dram_tensor("wproj", [256, C], F16, kind="ExternalInput").ap()
    wfc_d = nc.dram_tensor("wfc", [C, 4 * C], F16, kind="ExternalInput").ap()
    bfc_d = nc.dram_tensor("bfc", [4 * C], F32, kind="ExternalInput").ap()
    wmp_d = nc.dram_tensor("wmp", [4 * C, C], F16, kind="ExternalInput").ap()
    bmp_d = nc.dram_tensor("bmp", [C], F32, kind="ExternalInput").ap()
    ident_d = nc.dram_tensor("ident", [128, 128], F16, kind="ExternalInput").ap()
    gm_d = nc.dram_tensor("gm", [128, 128], F16, kind="ExternalInput").ap()
    xres_d = nc.dram_tensor("xres", [ROWS, C], F32, kind="ExternalInput").ap()
    out_d = nc.dram_tensor("out", [ROWS, C], F32, kind="ExternalOutput").ap()

    with tile.TileContext(nc) as tc:
        _body(nc, tc, locals())
    nc.compile()
    return nc


def _body(nc, tc, io):
    x_d = io["x_d"]; wqk_d = io["wqk_d"]; bqk_d = io["bqk_d"]; wv_d = io["wv_d"]
    wproj_d = io["wproj_d"]; wfc_d = io["wfc_d"]
    bfc_d = io["bfc_d"]; wmp_d = io["wmp_d"]; bmp_d = io["bmp_d"]
    ident_d = io["ident_d"]; gm_d = io["gm_d"]; xres_d = io["xres_d"]
    out_d = io["out_d"]

    AF = mybir.ActivationFunctionType
    OP = mybir.AluOpType

    consts = tc.alloc_tile_pool(name="consts", bufs=1)
    dram = tc.alloc_tile_pool(name="dram", bufs=1, space="DRAM")
    ps = tc.alloc_tile_pool(name="ps", bufs=6, space="PSUM")
    ps_av = tc.alloc_tile_pool(name="ps_av", bufs=2, space="PSUM")

    # ---------- constants (DMAs issued lazily below; tiles just declared) ----
    ident = consts.tile([128, 128], F16)
    gm = consts.tile([128, 128], F16)  # -30 above the causal diagonal
    epsb = consts.tile([128, 1], F32)
    bqk_sb = consts.tile([128, 4], F32)
    bfc_sb = consts.tile([128, 32], F32)
    bmp_bc = consts.tile([128, C], F32)
    ones_c = consts.tile([128, HPC, 1], F16)
    ones_b = consts.tile([128, 64], F16)

    # DRAM scratch (fp16 collective payload)
    attn_part = dram.tile([T, C], F16)
    rs_out = [dram.tile([128, C], F16, tag=f"rs{j}", name=f"rs{j}")
              for j in range(QC)]
    dnrm = [dram.tile([HPC, 512], F32, tag=f"dn{j}", name=f"dn{j}")
            for j in range(QC)]

    # ======== Pools (alloc order must honor LIFO release points) ========
    pEG = tc.alloc_tile_pool(name="pEG", bufs=1)   # x_mid (residual base)
    pEF = tc.alloc_tile_pool(name="pEF", bufs=1)   # h_ln^T
    stp2 = tc.alloc_tile_pool(name="stp2", bufs=4)
    xcp = tc.alloc_tile_pool(name="xcp", bufs=2)
    wfcp = tc.alloc_tile_pool(name="wfcp", bufs=16)
    pBC = tc.alloc_tile_pool(name="pBC", bufs=1)   # Q^T/K^T + V natural
    pCD = tc.alloc_tile_pool(name="pCD", bufs=1)   # y^T + w_proj
    probs = tc.alloc_tile_pool(name="probs", bufs=8)
    dsbp = tc.alloc_tile_pool(name="dsbp", bufs=4)
    ystg = tc.alloc_tile_pool(name="ystg", bufs=2)
    prst = tc.alloc_tile_pool(name="prst", bufs=3)
    pAB = tc.alloc_tile_pool(name="pAB", bufs=1)   # x_ln^T + qkv weights
    xpool = tc.alloc_tile_pool(name="xpool", bufs=3)
    stp = tc.alloc_tile_pool(name="stp", bufs=4)

    xlnT = pAB.tile([128, CB, T], F16, name="xlnT")
    wqk_sb = [pAB.tile([128, 512], F16, tag=f"wqk{i}", name=f"wqk{i}")
              for i in range(CB)]
    wv_sb = [pAB.tile([128, 256], F16, tag=f"wv{i}", name=f"wv{i}")
             for i in range(CB)]
    # Q^T per head, zero-padded to 128 rows (head's 64-row band at its
    # position in the K-pair tile; the other band is zero). Full-partition
    # streaming keeps the PE_HAM activity monitor at full clock during
    # scores (64-contract matmuls otherwise read as "idle" -> K=4/8).
    qz = [pBC.tile([128, T], F16, tag=f"qz{i}", name=f"qz{i}")
          for i in range(4)]
    kT = [pBC.tile([128, T], F16, tag=f"kT{i}", name=f"kT{i}")
          for i in range(2)]  # K^T, 2 heads stacked per tile
    vnat = [pBC.tile([128, 260], F16, tag=f"vnat{i}", name=f"vnat{i}")
            for i in range(TT)]  # per head: 64 V cols + ones col (65 each)
    yT = [pCD.tile([128, T], F16, tag=f"yT{i}", name=f"yT{i}")
          for i in range(2)]  # y^T, 2 heads per tile
    wproj_sb = [pCD.tile([128, C], F16, tag=f"wp{i}", name=f"wp{i}")
                for i in range(2)]
    x_mid = [pEG.tile([128, C], F32, tag=f"xmid{i}", name=f"xmid{i}")
             for i in range(QC)]
    hlnT = pEF.tile([128, CB, ROWS], F16, name="hlnT")

    def load_x_chunk(tcn):
        """Prefetch the 4 x tiles of a token chunk (fp16, 256KB each)."""
        xts = []
        for tt in range(4 * tcn, 4 * tcn + 4):
            xt = xpool.tile([128, C], F16, tag="xt", bufs=4)
            nc.sync.dma_start(out=xt, in_=x_d[tt * 128:(tt + 1) * 128, :])
            xts.append(xt)
        return xts

    def feed(tcn, xts):
        """LN1, transpose, V-natural, and qkv^T for chunk (x pre-fetched)."""
        for i4, tt in enumerate(range(4 * tcn, 4 * tcn + 4)):
            xt = xts[i4]
            st = stp.tile([128, 2, 6], F32, tag="st")
            xr = xt.rearrange("p (g f) -> p g f", g=2)
            nc.vector.bn_stats(out=st[:, 0, :], in_=xr[:, 0, :])
            nc.vector.bn_stats(out=st[:, 1, :], in_=xr[:, 1, :])
            mv = stp.tile([128, 2], F32, tag="mv")
            nc.vector.bn_aggr(out=mv, in_=st)
            rstd = stp.tile([128, 1], F32, tag="rstd")
            nc.scalar.activation(out=rstd, in_=mv[:, 1:2], func=AF.Sqrt,
                                 bias=epsb, scale=1.0)
            nc.vector.reciprocal(out=rstd, in_=rstd)
            xc = xpool.tile([128, C], F16, tag="xc", bufs=3)
            nc.vector.tensor_scalar(out=xc, in0=xt, scalar1=mv[:, 0:1],
                                    scalar2=rstd, op0=OP.subtract,
                                    op1=OP.mult)
            for cq in range(2):  # two psum banks of 4 transposes each
                pt = ps.tile([128, 512], F16, tag="mm", name="pt")
                for i in range(4):
                    cb = cq * 4 + i
                    nc.tensor.matmul(
                        pt[:, 128 * i:128 * (i + 1)],
                        xc[:, cb * 128:(cb + 1) * 128], ident,
                        is_transpose=True, start=(i == 0), stop=(i == 3))
                nc.scalar.copy(
                    out=xlnT[:, cq * 4:cq * 4 + 4, tt * 128:(tt + 1) * 128],
                    in_=pt.rearrange("p (i f) -> p i f", f=128))
            # V natural for this token tile
            pv = ps.tile([128, 256], F32, tag="mm", name="pv")
            for k in range(CB):
                nc.tensor.matmul(
                    pv, xlnT[:, k, tt * 128:(tt + 1) * 128],
                    wv_sb[k], start=(k == 0), stop=(k == CB - 1))
            nc.vector.tensor_copy(
                out=vnat[tt].rearrange("p (h x) -> p h x", x=65)[:, :, 64:65],
                in_=ones_c)
            nc.scalar.copy(
                out=vnat[tt].rearrange("p (h x) -> p h x", x=65)[:, :, 0:64],
                in_=pv.rearrange("p (h x) -> p h x", x=64))
        # Q^T/K^T columns for this token chunk
        cs = slice(tcn * 512, (tcn + 1) * 512)
        for mt in range(4):
            pq = ps.tile([128, 512], F32, tag="mm", name="pq")
            for k in range(CB):
                nc.tensor.matmul(
                    pq, wqk_sb[k][:, mt * 128:(mt + 1) * 128],
                    xlnT[:, k, tcn * 512:(tcn + 1) * 512],
                    start=(k == 0), stop=(k == CB - 1))
            if mt < 2:  # Q: split the head pair into the padded per-head tiles
                nc.scalar.activation(
                    out=qz[2 * mt][0:64, cs], in_=pq[0:64, :], func=AF.Identity,
                    bias=bqk_sb[0:64, mt:mt + 1], scale=1.0)
                nc.scalar.activation(
                    out=qz[2 * mt + 1][64:128, cs], in_=pq[64:128, :],
                    func=AF.Identity, bias=bqk_sb[64:128, mt:mt + 1], scale=1.0)
            else:
                nc.scalar.activation(
                    out=kT[mt - 2][:, cs], in_=pq, func=AF.Identity,
                    bias=bqk_sb[:, mt:mt + 1], scale=1.0)

    GRP = 4  # scores emitted in shape-uniform groups; PV trails one group

    def attention(qc, carry, heads):
        """carry: list of deferred (off, ysl, d16) normalizations."""
        for h in heads:
            off = 64 * (h % 2)
            qh = qz[h]   # 128 rows: head band + zeros
            kh = kT[h // 2]  # other head's rows hit Q's zero band
            nkb = 4 * qc + 4
            py = ps_av.tile([128, 512], F32, tag="py", name="py")
            pend = []
            for g0 in range(0, nkb, GRP):
                prs = []
                for kb in range(g0, min(g0 + GRP, nkb)):
                    j = kb - 4 * qc
                    lo = max(j, 0) * 128  # fully-masked columns skipped
                    pss = ps.tile([128, 512], F32, tag="mm", name="pss")
                    nc.tensor.matmul(
                        pss[:, lo:512], kh[:, kb * 128:(kb + 1) * 128],
                        qh[:, qc * 512 + lo:(qc + 1) * 512],
                        start=True, stop=(j < 0))
                    if j >= 0:  # causal diagonal: accumulate -30 above it
                        nc.tensor.matmul(
                            pss[:, lo:lo + 128], gm, ident,
                            start=False, stop=True, skip_group_check=True)
                    pr = probs.tile([128, 512], F16, tag="pr")
                    nc.scalar.activation(out=pr[:, lo:512],
                                         in_=pss[:, lo:512], func=AF.Exp)
                    prs.append((kb, lo, pr))
                if g0 == 0 and carry:
                    # one deferred y^T normalization per head (spacing)
                    _flush_one(carry.pop(0))
                for pkb, plo, ppr in pend:  # PV for the previous group
                    nc.tensor.matmul(
                        py[0:65, plo:512], vnat[pkb][:, h * 65:h * 65 + 65],
                        ppr[:, plo:512], start=(pkb == 0),
                        stop=(pkb == nkb - 1))
                pend = prs
            for pkb, plo, ppr in pend:
                nc.tensor.matmul(
                    py[0:65, plo:512], vnat[pkb][:, h * 65:h * 65 + 65],
                    ppr[:, plo:512], start=(pkb == 0), stop=(pkb == nkb - 1))
            # 1/denominator -> DRAM-bounced partition broadcast (deferred).
            # Staging copy on ScalarE; reciprocal reads SBUF (single-src
            # perf mode), keeping the PSUM-read penalty off VectorE.
            dsb = dsbp.tile([65, 512], F32, tag="dsb", bufs=2)
            nc.scalar.copy(out=dsb[64:65, :], in_=py[64:65, :])
            nc.vector.reciprocal(out=dsb[64:65, :], in_=dsb[64:65, :])
            nc.sync.dma_start(out=dnrm[qc][h, :], in_=dsb[64:65, :])
            rbc = dsbp.tile([64, 512], F32, tag="rbc", bufs=4)
            nc.sync.dma_start(out=rbc, in_=_bc(dnrm[qc][h, :], 64))
            ysl = yT[h // 2][off:off + 64, qc * 512:(qc + 1) * 512]
            carry.append((h, ysl, rbc, py))
        return carry

    def _flush_one(ent):
        # y^T = py * (1/den): single TT op reading the PV PSUM bank
        h0, ysl0, rbc0, py0 = ent
        if h0 % 2 == 0:
            nc.vector.tensor_mul(ysl0, py0[0:64, :], rbc0)
        else:
            yst = ystg.tile([64, 512], F16, tag="yst")
            nc.vector.tensor_mul(yst, py0[0:64, :], rbc0)
            nc.sync.dma_start(out=ysl0, in_=yst)

    def flush_norm(carry):
        for ent in carry:
            _flush_one(ent)
        carry.clear()

    def proj_rs(qc):
        for tt in range(4 * qc, 4 * qc + 4):
            for cc in range(2):
                pp = ps.tile([128, 512], F32, tag="mm", name="pp")
                for k in range(2):
                    nc.tensor.matmul(
                        pp, yT[k][:, tt * 128:(tt + 1) * 128],
                        wproj_sb[k][:, cc * 512:(cc + 1) * 512],
                        start=(k == 0), stop=(k == 1))
                pst = prst.tile([128, 512], F16, tag="pst")
                nc.vector.tensor_copy(out=pst, in_=pp)
                nc.sync.dma_start(
                    out=attn_part[tt * 128:(tt + 1) * 128,
                                  cc * 512:(cc + 1) * 512],
                    in_=pst)
        nc.gpsimd.collective_compute(
            "ReduceScatter", mybir.AluOpType.add, replica_groups=RG,
            ins=[attn_part[qc * 512:(qc + 1) * 512, :].opt()],
            outs=[rs_out[qc].opt()])

    def resid_ln2(qc, t_rst, t_rest):
        # residual + LN2 + h_ln^T for the owned 128-row strip.  The rst
        # load is hinted at the RS *trigger* time (so it lands on the
        # gpsimd queue before the next RS trigger); the compute chain is
        # hinted at the RS *completion* time (head-of-line avoidance on
        # the in-order vector queue).
        with tc.tile_wait_until(t_rst):
            xo = xcp.tile([128, C], F32, tag="xo")
            nc.sync.dma_start(out=xo, in_=xres_d[qc * 128:(qc + 1) * 128, :])
            rst = xcp.tile([128, C], F16, tag="rst")
            nc.gpsimd.dma_start(out=rst, in_=rs_out[qc])
        with tc.tile_wait_until(t_rest):
            _resid_ln2(qc, rst, xo)

    def _resid_ln2(qc, rst, xo):
        nc.vector.tensor_add(x_mid[qc], rst, xo)
        st = stp2.tile([128, 2, 6], F32, tag="st2")
        xr = x_mid[qc].rearrange("p (g f) -> p g f", g=2)
        nc.vector.bn_stats(out=st[:, 0, :], in_=xr[:, 0, :])
        nc.vector.bn_stats(out=st[:, 1, :], in_=xr[:, 1, :])
        mv = stp2.tile([128, 2], F32, tag="mv2")
        nc.vector.bn_aggr(out=mv, in_=st)
        rstd = stp2.tile([128, 1], F32, tag="rstd2")
        nc.scalar.activation(out=rstd, in_=mv[:, 1:2], func=AF.Sqrt,
                             bias=epsb, scale=1.0)
        nc.vector.reciprocal(out=rstd, in_=rstd)
        xc = xcp.tile([128, C], F16, tag="xc2")
        nc.vector.tensor_scalar(out=xc, in0=x_mid[qc], scalar1=mv[:, 0:1],
                                scalar2=rstd, op0=OP.subtract, op1=OP.mult)
        for cq in range(2):
            pt = ps.tile([128, 512], F16, tag="mm", name="pt2")
            for i in range(4):
                cb = cq * 4 + i
                nc.tensor.matmul(
                    pt[:, 128 * i:128 * (i + 1)],
                    xc[:, cb * 128:(cb + 1) * 128], ident,
                    is_transpose=True, start=(i == 0), stop=(i == 3))
            nc.vector.tensor_copy(
                out=hlnT[:, cq * 4:cq * 4 + 4, qc * 128:(qc + 1) * 128],
                in_=pt.rearrange("p (i f) -> p i f", f=128))

    def load_wfc_group(mg, tag2, t_load):
        wg = []
        with tc.tile_wait_until(t_load):
            for k in range(CB):
                w = wfcp.tile([128, 1024], F16, tag="wfc",
                              name=f"wfc{tag2}_{mg}_{k}")
                nc.scalar.dma_start(
                    out=w, in_=wfc_d[k * 128:(k + 1) * 128,
                                     mg * 1024:(mg + 1) * 1024])
                wg.append(w)
        return wg

    def fc_mg(mg, t0, t1, wg, h2gT):
        # h2^T = gelu(wfc^T @ h_ln^T + b_fc), one mg weight group, rows [t0,t1)
        n0, n1 = t0 * 128, t1 * 128
        for mt in range(8):
            m = mg * 8 + mt
            pf = ps.tile([128, 512], F32, tag="mm", name="pf")
            for k in range(CB):
                nc.tensor.matmul(
                    pf[:, 0:n1 - n0], wg[k][:, mt * 128:(mt + 1) * 128],
                    hlnT[:, k, n0:n1], start=(k == 0),
                    stop=(k == CB - 1))
            nc.scalar.activation(
                out=h2gT[:, m, n0:n1], in_=pf[:, 0:n1 - n0],
                func=getattr(AF, GELU_NAME),
                bias=bfc_sb[:, m:m + 1], scale=1.0)

    def g_pass(tlist, h2gT):
        # out rows = h2g^T.T @ wmp + x_mid for the given strips
        for cc in range(2):
            pg = {t: ps.tile([128, 512], F32, tag="mm", name=f"pg{cc}_{t}")
                  for t in tlist}
            for k in range(32):
                wm = wmpp.tile([128, 512], F16, tag="wmp")
                nc.sync.dma_start(
                    out=wm, in_=wmp_d[k * 128:(k + 1) * 128,
                                      cc * 512:(cc + 1) * 512])
                for t in tlist:
                    nc.tensor.matmul(
                        pg[t], h2gT[:, k, t * 128:(t + 1) * 128],
                        wm, start=(k == 0), stop=(k == 31))
            for t in tlist:
                ot = outp.tile([128, 512], F32, tag="ot")
                nc.vector.tensor_add(ot, pg[t],
                                     x_mid[t][:, cc * 512:(cc + 1) * 512])
                nc.vector.tensor_add(ot, ot,
                                     bmp_bc[:, cc * 512:(cc + 1) * 512])
                nc.sync.dma_start(
                    out=out_d[t * 128:(t + 1) * 128,
                              cc * 512:(cc + 1) * 512],
                    in_=ot)

    # ====== unified software pipeline over token/query chunks ======
    # DMA issue order = priority: x chunk 0 first, then ident (transposes),
    # wv/wqk (feed 0), trim (attn 0); heavier / later-needed loads follow.
    xts0 = load_x_chunk(0)
    nc.sync.dma_start(out=ident, in_=ident_d)
    nc.vector.memset(epsb, EPS)
    nc.vector.memset(ones_c, 1.0)
    nc.vector.memset(ones_b, 1.0)
    for h in range(4):  # zero the unused 64-row band of each padded Q^T
        zoff = 0 if h % 2 else 64
        nc.vector.memset(qz[h][zoff:zoff + 64, :], 0.0)
    for k in range(CB):
        nc.sync.dma_start(out=wv_sb[k], in_=wv_d[k * 128:(k + 1) * 128, :])
    for k in range(CB):
        nc.sync.dma_start(out=wqk_sb[k], in_=wqk_d[k * 128:(k + 1) * 128, :])
    nc.sync.dma_start(out=bqk_sb, in_=bqk_d.rearrange("(m p) -> p m", p=128))
    nc.sync.dma_start(out=gm, in_=gm_d)
    xts1 = load_x_chunk(1)
    for k in range(2):
        nc.sync.dma_start(out=wproj_sb[k],
                          in_=wproj_d[k * 128:(k + 1) * 128, :])

    carry = []
    feed(0, xts0)
    # deferred const loads (needed from resid_ln2 / fc onwards)
    nc.sync.dma_start(out=bmp_bc, in_=_bc(bmp_d, 128))
    nc.sync.dma_start(out=bfc_sb, in_=bfc_d.rearrange("(m p) -> p m", p=128))
    feed(1, xts1)
    xts2 = load_x_chunk(2)
    xts3 = load_x_chunk(3)
    feed(2, xts2)
    feed(3, xts3)
    stp.release()
    xpool.release()
    pAB.release()
    # MLP pools open as soon as the feed pools close (xlnT freed); strips
    # are processed newest-first so the cheap chunk-0 RS lands last.
    pFG = tc.alloc_tile_pool(name="pFG", bufs=1)   # gelu(h2)^T
    wmpp = tc.alloc_tile_pool(name="wmpp", bufs=8)
    outp = tc.alloc_tile_pool(name="outp", bufs=3)
    h2gT = pFG.tile([128, 32, ROWS], F16, name="h2gT")

    T_RST = {3: 0.152, 2: 0.192, 1: 0.217, 0: 0.242}
    T_LN2 = {3: 0.180, 2: 0.220, 1: 0.247, 0: 0.272}
    for qc in (3, 2, 1, 0):
        attention(qc, carry, range(HPC))
        flush_norm(carry)
        proj_rs(qc)
        resid_ln2(qc, T_RST[qc], T_LN2[qc])

    # fc wave A: strips 2,3 (ready first); wave B: strips 0,1
    wgs = {mg: load_wfc_group(mg, "a", 0.160 + 0.012 * mg)
           for mg in range(4)}
    for mg in range(4):
        with tc.tile_wait_until(0.224 + 0.004 * mg):
            fc_mg(mg, 2, 4, wgs[mg], h2gT)
    with tc.tile_wait_until(0.274):
        fc_mg(3, 0, 2, wgs[3], h2gT)
        fc_mg(2, 0, 2, wgs[2], h2gT)
        fc_mg(1, 0, 2, load_wfc_group(1, "b", 0.250), h2gT)
        fc_mg(0, 0, 2, load_wfc_group(0, "b", 0.256), h2gT)
    with tc.tile_wait_until(0.260):
        g_pass([2, 3], h2gT)
    with tc.tile_wait_until(0.310):
        g_pass([0, 1], h2gT)

    outp.release()
    wmpp.release()
    pFG.release()
    prst.release()
    ystg.release()
    dsbp.release()
    probs.release()
    pCD.release()
    pBC.release()
    wfcp.release()
    xcp.release()
    stp2.release()
    pEF.release()
    pEG.release()
    ps_av.release()
    ps.release()
    dram.release()
    consts.release()



_CACHED = None


def _get_program():
    global _CACHED
    if _CACHED is None:
        _CACHED = build_program()
    return _CACHED


def _prep_inputs(inputs):
    """Fold LN params into weights and build the 8 per-core input maps."""
    x = np.asarray(inputs["x"], np.float32)
    ln1_w = np.asarray(inputs["ln1_w"], np.float32)
    ln1_b = np.asarray(inputs["ln1_b"], np.float32)
    w_attn = np.asarray(inputs["w_attn"], np.float32)
    b_attn = np.asarray(inputs["b_attn"], np.float32)
    w_proj = np.asarray(inputs["w_proj"], np.float32)
    b_proj = np.asarray(inputs["b_proj"], np.float32)
    ln2_w = np.asarray(inputs["ln2_w"], np.float32)
    ln2_b = np.asarray(inputs["ln2_b"], np.float32)
    w_fc = np.asarray(inputs["w_fc"], np.float32)
    b_fc = np.asarray(inputs["b_fc"], np.float32)
    w_mp = np.asarray(inputs["w_mlp_proj"], np.float32)
    b_mp = np.asarray(inputs["b_mlp_proj"], np.float32)

    Wa = ln1_w[:, None] * w_attn                      # [C, 3C]
    Ba = b_attn + ln1_b @ w_attn                      # [3C]
    s = 1.0 / np.sqrt(D)
    Wq = Wa[:, 0:C] * s
    Bq = Ba[0:C] * s
    Wk = Wa[:, C:2 * C]
    Bk = Ba[C:2 * C]
    Wv = Wa[:, 2 * C:3 * C]
    Bv = Ba[2 * C:3 * C]
    bproj_eff = (b_proj + Bv @ w_proj).astype(np.float32)

    Wfc = (ln2_w[:, None] * w_fc).astype(np.float32)
    Bfc = (b_fc + ln2_b @ w_fc).astype(np.float32)

    ident = np.eye(128, dtype=np.float16)
    gm = np.where(np.arange(128)[:, None] < np.arange(128)[None, :],
                  np.float16(-30.0), np.float16(0.0))

    in_maps = []
    for c in range(N_CORES):
        g, p = divmod(c, TP)
        hs = slice(HPC * D * p, HPC * D * (p + 1))    # 256 cols/rows per core
        wqk = np.ascontiguousarray(
            np.concatenate([Wq[:, hs], Wk[:, hs]], axis=1), np.float16)
        bqk = np.ascontiguousarray(
            np.concatenate([Bq[hs], Bk[hs]]), np.float32)
        xres = np.concatenate(
            [x[g][512 * j + 128 * p:512 * j + 128 * p + 128]
             for j in range(QC)], axis=0) + bproj_eff[None, :]
        in_maps.append({
            "x": np.ascontiguousarray(x[g]).astype(np.float16),
            "xres": np.ascontiguousarray(xres),
            "wqk": wqk,
            "bqk": bqk,
            "wv": np.ascontiguousarray(Wv[:, hs]).astype(np.float16),
            "wproj": np.ascontiguousarray(w_proj[hs, :]).astype(np.float16),
            "wfc": Wfc.astype(np.float16),
            "bfc": Bfc,
            "wmp": w_mp.astype(np.float16),
            "bmp": b_mp,
            "ident": ident,
            "gm": gm,
        })
    return in_maps


def _gather(results):
    out = np.empty((B, T, C), np.float32)
    for c in range(N_CORES):
        g, p = divmod(c, TP)
        for j in range(QC):
            out[g, 512 * j + 128 * p:512 * j + 128 * p + 128, :] = \
                results[c]["out"][128 * j:128 * (j + 1)]
    return out


def kernel(**inputs) -> np.ndarray:
    nc = _get_program()
    in_maps = _prep_inputs(inputs)
    res = run_bass_kernel_spmd(nc, in_maps, list(range(N_CORES)))
    return _gather(res.results)


if __name__ == "__main__":
    print("building program...")
    _get_program()
    print("built ok")


# revision 27
# speedup vs baseline: 1.2846x; 1.2846x over previous
"""Trainium2 Bass kernel for a dense transformer block (B=2, T=2048, C=1024, H=16).

Sharding: DP2 (batch -> core groups {0-3},{4-7}) x TP4 within a group:
  - attention: Megatron head-parallel (4 heads/core), row-parallel out-proj,
    pipelined ReduceScatter(add) over the group (one RS per 512-row block,
    issued immediately after that query-chunk's attention).
  - MLP: sequence-parallel (each core computes its 512 rows with the FULL
    fc / proj weights). No other collective.

Row ownership: core at group position p owns rows {512j+128p .. 512j+128p+128}
for j in 0..3 (one 128-row strip per pipelined ReduceScatter).

Device layout notes:
  - Activations feeding matmuls are kept transposed [features, tokens]
    ("^T layout") so every matmul contracts over the partition dim.
  - LN affine params are folded into the following weights on the host;
    q-scale (1/sqrt(D)) folded into W_q/b_q; v-bias folded into b_proj.
  - Softmax: scores^T[k,q] tiles; exp on ScalarE (no max subtraction:
    scores are ~N(0,1), safe); denominator via ones-column appended to V
    (row 64 of the PV matmul output); normalization applied to y^T with a
    DRAM-bounced partition-broadcast of 1/denom.
  - Matmul operands are fp16 (full PE rate, fast weight load); all
    accumulation, softmax statistics, residuals and LN are fp32.
  - x arrives fp16 (host-cast) to cut first-tile DMA latency; the fp32
    residual base (xres) is a separate input.
  - Scheduling: proj+RS issued per chunk right after its attention; MLP
    weights streamed once (strip-3 fc wave reuses the 2 resident groups);
    single wmp stream covers all 4 strips.
"""

import os
import sys

import numpy as np

for _p in ("/opt/trn_rl_repo", "/root/.axon_site/_ro/trn_rl_repo"):
    if os.path.isdir(_p) and _p not in sys.path:
        sys.path.insert(0, _p)

import concourse.bass as bass
import concourse.tile as tile
from concourse import bacc, mybir
from concourse.bass_utils import run_bass_kernel_spmd

B, T, C, H = 2, 2048, 1024, 16
D = C // H  # 64
EPS = 1e-5
N_CORES = 8
TP = 4            # tensor-parallel group size
HPC = 4           # heads per core
ROWS = T // TP    # 512 token rows owned per core
F32 = mybir.dt.float32
F16 = mybir.dt.float16  # matmul operand dtype

TT = T // 128     # 16 token tiles
CB = C // 128     # 8 channel blocks
QC = T // 512     # 4 query chunks / row blocks
RG = [[0, 1, 2, 3], [4, 5, 6, 7]]

GELU_NAME = "Gelu_apprx_tanh"  # sim_check overrides (sim lacks Gelu)


def _bc(ap, p):
    """Broadcast a DRAM AP across p partitions (prepend stride-0 dim)."""
    return bass.AP(tensor=ap.tensor, offset=ap.offset, ap=[[0, p], *ap.ap])


def build_program():
    nc = bacc.Bacc(
        "TRN2", target_bir_lowering=False, debug=False, num_devices=N_CORES
    )

    # ---- I/O ----
    x_d = nc.dram_tensor("x", [T, C], F16, kind="ExternalInput").ap()
    wqk_d = nc.dram_tensor("wqk", [C, 512], F16, kind="ExternalInput").ap()
    bqk_d = nc.dram_tensor("bqk", [512], F32, kind="ExternalInput").ap()
    wv_d = nc.dram_tensor("wv", [C, 256], F16, kind="ExternalInput").ap()
    wproj_d = nc.dram_tensor("wproj", [256, C], F16, kind="ExternalInput").ap()
    wfc_d = nc.dram_tensor("wfc", [C, 4 * C], F16, kind="ExternalInput").ap()
    bfc_d = nc.dram_tensor("bfc", [4 * C], F32, kind="ExternalInput").ap()
    wmp_d = nc.dram_tensor("wmp", [4 * C, C], F16, kind="ExternalInput").ap()
    bmp_d = nc.dram_tensor("bmp", [C], F32, kind="ExternalInput").ap()
    ident_d = nc.dram_tensor("ident", [128, 128], F16, kind="ExternalInput").ap()
    gm_d = nc.dram_tensor("gm", [128, 128], F16, kind="ExternalInput").ap()
    xres_d = nc.dram_tensor("xres", [ROWS, C], F32, kind="ExternalInput").ap()
    out_d = nc.dram_tensor("out", [ROWS, C], F32, kind="ExternalOutput").ap()

    with tile.TileContext(nc) as tc:
        _body(nc, tc, locals())
    nc.compile()
    return nc


def _body(nc, tc, io):
    x_d = io["x_d"]; wqk_d = io["wqk_d"]; bqk_d = io["bqk_d"]; wv_d = io["wv_d"]
    wproj_d = io["wproj_d"]; wfc_d = io["wfc_d"]
    bfc_d = io["bfc_d"]; wmp_d = io["wmp_d"]; bmp_d = io["bmp_d"]
    ident_d = io["ident_d"]; gm_d = io["gm_d"]; xres_d = io["xres_d"]
    out_d = io["out_d"]

    AF = mybir.ActivationFunctionType
    OP = mybir.AluOpType

    consts = tc.alloc_tile_pool(name="consts", bufs=1)
    dram = tc.alloc_tile_pool(name="dram", bufs=1, space="DRAM")
    ps = tc.alloc_tile_pool(name="ps", bufs=6, space="PSUM")
    ps_av = tc.alloc_tile_pool(name="ps_av", bufs=2, space="PSUM")

    # ---------- constants (DMAs issued lazily below; tiles just declared) ----
    ident = consts.tile([128, 128], F16)
    gm = consts.tile([128, 128], F16)  # -30 above the causal diagonal
    epsb = consts.tile([128, 1], F32)
    bqk_sb = consts.tile([128, 4], F32)
    bfc_sb = consts.tile([128, 32], F32)
    bmp_bc = consts.tile([128, C], F32)
    ones_c = consts.tile([128, HPC, 1], F16)
    ones_b = consts.tile([128, 64], F16)

    # DRAM scratch (fp16 collective payload)
    attn_part = dram.tile([T, C], F16)
    rs_out = [dram.tile([128, C], F16, tag=f"rs{j}", name=f"rs{j}")
              for j in range(QC)]
    dnrm = [dram.tile([HPC, 512], F32, tag=f"dn{j}", name=f"dn{j}")
            for j in range(QC)]

    # ======== Pools (alloc order must honor LIFO release points) ========
    pEG = tc.alloc_tile_pool(name="pEG", bufs=1)   # x_mid (residual base)
    pEF = tc.alloc_tile_pool(name="pEF", bufs=1)   # h_ln^T
    stp2 = tc.alloc_tile_pool(name="stp2", bufs=4)
    xcp = tc.alloc_tile_pool(name="xcp", bufs=2)
    wfcp = tc.alloc_tile_pool(name="wfcp", bufs=16)
    pBC = tc.alloc_tile_pool(name="pBC", bufs=1)   # Q^T/K^T + V natural
    pCD = tc.alloc_tile_pool(name="pCD", bufs=1)   # y^T + w_proj
    probs = tc.alloc_tile_pool(name="probs", bufs=8)
    dsbp = tc.alloc_tile_pool(name="dsbp", bufs=4)
    ystg = tc.alloc_tile_pool(name="ystg", bufs=2)
    prst = tc.alloc_tile_pool(name="prst", bufs=3)
    pAB = tc.alloc_tile_pool(name="pAB", bufs=1)   # x_ln^T + qkv weights
    xpool = tc.alloc_tile_pool(name="xpool", bufs=3)
    stp = tc.alloc_tile_pool(name="stp", bufs=4)

    xlnT = pAB.tile([128, CB, T], F16, name="xlnT")
    wqk_sb = [pAB.tile([128, 512], F16, tag=f"wqk{i}", name=f"wqk{i}")
              for i in range(CB)]
    wv_sb = [pAB.tile([128, 256], F16, tag=f"wv{i}", name=f"wv{i}")
             for i in range(CB)]
    # Q^T per head, zero-padded to 128 rows (head's 64-row band at its
    # position in the K-pair tile; the other band is zero). Full-partition
    # streaming keeps the PE_HAM activity monitor at full clock during
    # scores (64-contract matmuls otherwise read as "idle" -> K=4/8).
    qz = [pBC.tile([128, T], F16, tag=f"qz{i}", name=f"qz{i}")
          for i in range(4)]
    kT = [pBC.tile([128, T], F16, tag=f"kT{i}", name=f"kT{i}")
          for i in range(2)]  # K^T, 2 heads stacked per tile
    vnat = [pBC.tile([128, 260], F16, tag=f"vnat{i}", name=f"vnat{i}")
            for i in range(TT)]  # per head: 64 V cols + ones col (65 each)
    yT = [pCD.tile([128, T], F16, tag=f"yT{i}", name=f"yT{i}")
          for i in range(2)]  # y^T, 2 heads per tile
    wproj_sb = [pCD.tile([128, C], F16, tag=f"wp{i}", name=f"wp{i}")
                for i in range(2)]
    x_mid = [pEG.tile([128, C], F32, tag=f"xmid{i}", name=f"xmid{i}")
             for i in range(QC)]
    hlnT = pEF.tile([128, CB, ROWS], F16, name="hlnT")

    def load_x_chunk(tcn):
        """Prefetch the 4 x tiles of a token chunk (fp16, 256KB each)."""
        xts = []
        for tt in range(4 * tcn, 4 * tcn + 4):
            xt = xpool.tile([128, C], F16, tag="xt", bufs=4)
            nc.sync.dma_start(out=xt, in_=x_d[tt * 128:(tt + 1) * 128, :])
            xts.append(xt)
        return xts

    def feed(tcn, xts):
        """LN1, transpose, V-natural, and qkv^T for chunk (x pre-fetched)."""
        for i4, tt in enumerate(range(4 * tcn, 4 * tcn + 4)):
            xt = xts[i4]
            st = stp.tile([128, 2, 6], F32, tag="st")
            xr = xt.rearrange("p (g f) -> p g f", g=2)
            nc.vector.bn_stats(out=st[:, 0, :], in_=xr[:, 0, :])
            nc.vector.bn_stats(out=st[:, 1, :], in_=xr[:, 1, :])
            mv = stp.tile([128, 2], F32, tag="mv")
            nc.vector.bn_aggr(out=mv, in_=st)
            rstd = stp.tile([128, 1], F32, tag="rstd")
            nc.scalar.activation(out=rstd, in_=mv[:, 1:2], func=AF.Sqrt,
                                 bias=epsb, scale=1.0)
            nc.vector.reciprocal(out=rstd, in_=rstd)
            xc = xpool.tile([128, C], F16, tag="xc", bufs=3)
            nc.vector.tensor_scalar(out=xc, in0=xt, scalar1=mv[:, 0:1],
                                    scalar2=rstd, op0=OP.subtract,
                                    op1=OP.mult)
            for cq in range(2):  # two psum banks of 4 transposes each
                pt = ps.tile([128, 512], F16, tag="mm", name="pt")
                for i in range(4):
                    cb = cq * 4 + i
                    nc.tensor.matmul(
                        pt[:, 128 * i:128 * (i + 1)],
                        xc[:, cb * 128:(cb + 1) * 128], ident,
                        is_transpose=True, start=(i == 0), stop=(i == 3))
                nc.vector.tensor_copy(
                    out=xlnT[:, cq * 4:cq * 4 + 4, tt * 128:(tt + 1) * 128],
                    in_=pt.rearrange("p (i f) -> p i f", f=128))
            # V natural for this token tile
            pv = ps.tile([128, 256], F32, tag="mm", name="pv")
            for k in range(CB):
                nc.tensor.matmul(
                    pv, xlnT[:, k, tt * 128:(tt + 1) * 128],
                    wv_sb[k], start=(k == 0), stop=(k == CB - 1))
            nc.vector.tensor_copy(
                out=vnat[tt].rearrange("p (h x) -> p h x", x=65)[:, :, 64:65],
                in_=ones_c)
            nc.vector.tensor_copy(
                out=vnat[tt].rearrange("p (h x) -> p h x", x=65)[:, :, 0:64],
                in_=pv.rearrange("p (h x) -> p h x", x=64))
        # Q^T/K^T columns for this token chunk
        cs = slice(tcn * 512, (tcn + 1) * 512)
        for mt in range(4):
            pq = ps.tile([128, 512], F32, tag="mm", name="pq")
            for k in range(CB):
                nc.tensor.matmul(
                    pq, wqk_sb[k][:, mt * 128:(mt + 1) * 128],
                    xlnT[:, k, tcn * 512:(tcn + 1) * 512],
                    start=(k == 0), stop=(k == CB - 1))
            if mt < 2:  # Q: split the head pair into the padded per-head tiles
                nc.vector.tensor_scalar_add(
                    out=qz[2 * mt][0:64, cs], in0=pq[0:64, :],
                    scalar1=bqk_sb[0:64, mt:mt + 1])
                nc.vector.tensor_scalar_add(
                    out=qz[2 * mt + 1][64:128, cs], in0=pq[64:128, :],
                    scalar1=bqk_sb[64:128, mt:mt + 1])
            else:
                nc.vector.tensor_scalar_add(
                    out=kT[mt - 2][:, cs], in0=pq,
                    scalar1=bqk_sb[:, mt:mt + 1])

    GRP = 4  # scores emitted in shape-uniform groups; PV trails one group

    def attention(qc, carry, heads):
        """carry: list of deferred (off, ysl, d16) normalizations."""
        for h in heads:
            off = 64 * (h % 2)
            qh = qz[h]   # 128 rows: head band + zeros
            kh = kT[h // 2]  # other head's rows hit Q's zero band
            nkb = 4 * qc + 4
            py = ps_av.tile([128, 512], F32, tag="py", name="py")
            pend = []
            for g0 in range(0, nkb, GRP):
                prs = []
                for kb in range(g0, min(g0 + GRP, nkb)):
                    j = kb - 4 * qc
                    lo = max(j, 0) * 128  # fully-masked columns skipped
                    pss = ps.tile([128, 512], F32, tag="mm", name="pss")
                    nc.tensor.matmul(
                        pss[:, lo:512], kh[:, kb * 128:(kb + 1) * 128],
                        qh[:, qc * 512 + lo:(qc + 1) * 512],
                        start=True, stop=(j < 0))
                    if j >= 0:  # causal diagonal: accumulate -30 above it
                        nc.tensor.matmul(
                            pss[:, lo:lo + 128], gm, ident,
                            start=False, stop=True, skip_group_check=True)
                    pr = probs.tile([128, 512], F16, tag="pr")
                    nc.scalar.activation(out=pr[:, lo:512],
                                         in_=pss[:, lo:512], func=AF.Exp)
                    prs.append((kb, lo, pr))
                if g0 == 0 and carry:
                    # one deferred y^T normalization per head (spacing)
                    _flush_one(carry.pop(0))
                for pkb, plo, ppr in pend:  # PV for the previous group
                    nc.tensor.matmul(
                        py[0:65, plo:512], vnat[pkb][:, h * 65:h * 65 + 65],
                        ppr[:, plo:512], start=(pkb == 0),
                        stop=(pkb == nkb - 1))
                pend = prs
            for pkb, plo, ppr in pend:
                nc.tensor.matmul(
                    py[0:65, plo:512], vnat[pkb][:, h * 65:h * 65 + 65],
                    ppr[:, plo:512], start=(pkb == 0), stop=(pkb == nkb - 1))
            # 1/denominator -> DRAM-bounced partition broadcast (deferred).
            # Staging copy on ScalarE; reciprocal reads SBUF (single-src
            # perf mode), keeping the PSUM-read penalty off VectorE.
            dsb = dsbp.tile([65, 512], F32, tag="dsb", bufs=2)
            nc.scalar.copy(out=dsb[64:65, :], in_=py[64:65, :])
            nc.vector.reciprocal(out=dsb[64:65, :], in_=dsb[64:65, :])
            nc.sync.dma_start(out=dnrm[qc][h, :], in_=dsb[64:65, :])
            rbc = dsbp.tile([64, 512], F32, tag="rbc", bufs=4)
            nc.sync.dma_start(out=rbc, in_=_bc(dnrm[qc][h, :], 64))
            ysl = yT[h // 2][off:off + 64, qc * 512:(qc + 1) * 512]
            carry.append((h, ysl, rbc, py))
        return carry

    def _flush_one(ent):
        # y^T = py * (1/den): single TT op reading the PV PSUM bank
        h0, ysl0, rbc0, py0 = ent
        if h0 % 2 == 0:
            nc.vector.tensor_mul(ysl0, py0[0:64, :], rbc0)
        else:
            yst = ystg.tile([64, 512], F16, tag="yst")
            nc.vector.tensor_mul(yst, py0[0:64, :], rbc0)
            nc.sync.dma_start(out=ysl0, in_=yst)

    def flush_norm(carry):
        for ent in carry:
            _flush_one(ent)
        carry.clear()

    def proj_rs(qc):
        for tt in range(4 * qc, 4 * qc + 4):
            for cc in range(2):
                pp = ps.tile([128, 512], F32, tag="mm", name="pp")
                for k in range(2):
                    nc.tensor.matmul(
                        pp, yT[k][:, tt * 128:(tt + 1) * 128],
                        wproj_sb[k][:, cc * 512:(cc + 1) * 512],
                        start=(k == 0), stop=(k == 1))
                pst = prst.tile([128, 512], F16, tag="pst")
                nc.vector.tensor_copy(out=pst, in_=pp)
                nc.sync.dma_start(
                    out=attn_part[tt * 128:(tt + 1) * 128,
                                  cc * 512:(cc + 1) * 512],
                    in_=pst)
        nc.gpsimd.collective_compute(
            "ReduceScatter", mybir.AluOpType.add, replica_groups=RG,
            ins=[attn_part[qc * 512:(qc + 1) * 512, :].opt()],
            outs=[rs_out[qc].opt()])

    def resid_ln2(qc, t_rst, t_rest):
        # residual + LN2 + h_ln^T for the owned 128-row strip.  The rst
        # load is hinted at the RS *trigger* time (so it lands on the
        # gpsimd queue before the next RS trigger); the compute chain is
        # hinted at the RS *completion* time (head-of-line avoidance on
        # the in-order vector queue).
        with tc.tile_wait_until(t_rst):
            xo = xcp.tile([128, C], F32, tag="xo")
            nc.sync.dma_start(out=xo, in_=xres_d[qc * 128:(qc + 1) * 128, :])
            rst = xcp.tile([128, C], F16, tag="rst")
            nc.gpsimd.dma_start(out=rst, in_=rs_out[qc])
        with tc.tile_wait_until(t_rest):
            _resid_ln2(qc, rst, xo)

    def _resid_ln2(qc, rst, xo):
        nc.vector.tensor_add(x_mid[qc], rst, xo)
        st = stp2.tile([128, 2, 6], F32, tag="st2")
        xr = x_mid[qc].rearrange("p (g f) -> p g f", g=2)
        nc.vector.bn_stats(out=st[:, 0, :], in_=xr[:, 0, :])
        nc.vector.bn_stats(out=st[:, 1, :], in_=xr[:, 1, :])
        mv = stp2.tile([128, 2], F32, tag="mv2")
        nc.vector.bn_aggr(out=mv, in_=st)
        rstd = stp2.tile([128, 1], F32, tag="rstd2")
        nc.scalar.activation(out=rstd, in_=mv[:, 1:2], func=AF.Sqrt,
                             bias=epsb, scale=1.0)
        nc.vector.reciprocal(out=rstd, in_=rstd)
        xc = xcp.tile([128, C], F16, tag="xc2")
        nc.vector.tensor_scalar(out=xc, in0=x_mid[qc], scalar1=mv[:, 0:1],
                                scalar2=rstd, op0=OP.subtract, op1=OP.mult)
        for cq in range(2):
            pt = ps.tile([128, 512], F16, tag="mm", name="pt2")
            for i in range(4):
                cb = cq * 4 + i
                nc.tensor.matmul(
                    pt[:, 128 * i:128 * (i + 1)],
                    xc[:, cb * 128:(cb + 1) * 128], ident,
                    is_transpose=True, start=(i == 0), stop=(i == 3))
            nc.vector.tensor_copy(
                out=hlnT[:, cq * 4:cq * 4 + 4, qc * 128:(qc + 1) * 128],
                in_=pt.rearrange("p (i f) -> p i f", f=128))

    def load_wfc_group(mg, tag2, t_load):
        wg = []
        with tc.tile_wait_until(t_load):
            for k in range(CB):
                w = wfcp.tile([128, 1024], F16, tag="wfc",
                              name=f"wfc{tag2}_{mg}_{k}")
                nc.scalar.dma_start(
                    out=w, in_=wfc_d[k * 128:(k + 1) * 128,
                                     mg * 1024:(mg + 1) * 1024])
                wg.append(w)
        return wg

    def fc_mg(mg, t0, t1, wg, h2gT):
        # h2^T = gelu(wfc^T @ h_ln^T + b_fc), one mg weight group, rows [t0,t1)
        n0, n1 = t0 * 128, t1 * 128
        for mt in range(8):
            m = mg * 8 + mt
            pf = ps.tile([128, 512], F32, tag="mm", name="pf")
            for k in range(CB):
                nc.tensor.matmul(
                    pf[:, 0:n1 - n0], wg[k][:, mt * 128:(mt + 1) * 128],
                    hlnT[:, k, n0:n1], start=(k == 0),
                    stop=(k == CB - 1))
            nc.scalar.activation(
                out=h2gT[:, m, n0:n1], in_=pf[:, 0:n1 - n0],
                func=getattr(AF, GELU_NAME),
                bias=bfc_sb[:, m:m + 1], scale=1.0)

    def g_pass(tlist, h2gT):
        # out rows = h2g^T.T @ wmp + x_mid for the given strips
        for cc in range(2):
            pg = {t: ps.tile([128, 512], F32, tag="mm", name=f"pg{cc}_{t}")
                  for t in tlist}
            for k in range(32):
                wm = wmpp.tile([128, 512], F16, tag="wmp")
                nc.sync.dma_start(
                    out=wm, in_=wmp_d[k * 128:(k + 1) * 128,
                                      cc * 512:(cc + 1) * 512])
                for t in tlist:
                    nc.tensor.matmul(
                        pg[t], h2gT[:, k, t * 128:(t + 1) * 128],
                        wm, start=(k == 0), stop=(k == 31))
            for t in tlist:
                ot = outp.tile([128, 512], F32, tag="ot")
                nc.vector.tensor_add(ot, pg[t],
                                     x_mid[t][:, cc * 512:(cc + 1) * 512])
                nc.vector.tensor_add(ot, ot,
                                     bmp_bc[:, cc * 512:(cc + 1) * 512])
                nc.sync.dma_start(
                    out=out_d[t * 128:(t + 1) * 128,
                              cc * 512:(cc + 1) * 512],
                    in_=ot)

    # ====== unified software pipeline over token/query chunks ======
    # DMA issue order = priority: x chunk 0 first, then ident (transposes),
    # wv/wqk (feed 0), trim (attn 0); heavier / later-needed loads follow.
    xts0 = load_x_chunk(0)
    nc.sync.dma_start(out=ident, in_=ident_d)
    nc.vector.memset(epsb, EPS)
    nc.vector.memset(ones_c, 1.0)
    nc.vector.memset(ones_b, 1.0)
    for h in range(4):  # zero the unused 64-row band of each padded Q^T
        zoff = 0 if h % 2 else 64
        nc.vector.memset(qz[h][zoff:zoff + 64, :], 0.0)
    for k in range(CB):
        nc.sync.dma_start(out=wv_sb[k], in_=wv_d[k * 128:(k + 1) * 128, :])
    for k in range(CB):
        nc.sync.dma_start(out=wqk_sb[k], in_=wqk_d[k * 128:(k + 1) * 128, :])
    nc.sync.dma_start(out=bqk_sb, in_=bqk_d.rearrange("(m p) -> p m", p=128))
    nc.sync.dma_start(out=gm, in_=gm_d)
    xts1 = load_x_chunk(1)
    for k in range(2):
        nc.sync.dma_start(out=wproj_sb[k],
                          in_=wproj_d[k * 128:(k + 1) * 128, :])

    carry = []
    feed(0, xts0)
    attention(0, carry, range(HPC))
    xts2 = load_x_chunk(2)
    # deferred const loads (needed from resid_ln2 / fc onwards)
    nc.sync.dma_start(out=bmp_bc, in_=_bc(bmp_d, 128))
    nc.sync.dma_start(out=bfc_sb, in_=bfc_d.rearrange("(m p) -> p m", p=128))
    feed(1, xts1)
    flush_norm(carry)
    proj_rs(0)
    attention(1, carry, range(HPC))
    xts3 = load_x_chunk(3)
    feed(2, xts2)
    flush_norm(carry)
    proj_rs(1)
    resid_ln2(0, 0.100, 0.130)
    attention(2, carry, range(HPC))
    feed(3, xts3)
    flush_norm(carry)
    proj_rs(2)
    resid_ln2(1, 0.158, 0.180)
    attention(3, carry, range(HPC))
    flush_norm(carry)
    resid_ln2(2, 0.232, 0.252)
    proj_rs(3)
    stp.release()
    xpool.release()
    pAB.release()
    # MLP pools open only after the attention pools close (SBUF budget)
    pFG = tc.alloc_tile_pool(name="pFG", bufs=1)   # gelu(h2)^T
    wmpp = tc.alloc_tile_pool(name="wmpp", bufs=8)
    outp = tc.alloc_tile_pool(name="outp", bufs=3)
    h2gT = pFG.tile([128, 32, ROWS], F16, name="h2gT")
    # fc wave 1: strips 0-2; wfc groups streamed once (bufs=16 -> the last
    # two groups stay resident for the strip-3 wave)
    wgs = {mg: load_wfc_group(mg, "a", 0.200 + 0.012 * mg)
           for mg in range(4)}
    for mg in range(4):
        with tc.tile_wait_until(0.256 + 0.004 * mg):
            fc_mg(mg, 0, 3, wgs[mg], h2gT)
    resid_ln2(3, 0.302, 0.333)
    # g wave 1 (strips 0-2) runs while the strip-3 RS/fc tail resolves
    with tc.tile_wait_until(0.315):
        g_pass([0, 1, 2], h2gT)
    with tc.tile_wait_until(0.345):
        fc_mg(3, 3, 4, wgs[3], h2gT)
        fc_mg(2, 3, 4, wgs[2], h2gT)
        fc_mg(1, 3, 4, load_wfc_group(1, "b", 0.300), h2gT)
        fc_mg(0, 3, 4, load_wfc_group(0, "b", 0.306), h2gT)
    with tc.tile_wait_until(0.380):
        g_pass([3], h2gT)

    outp.release()
    wmpp.release()
    pFG.release()
    prst.release()
    ystg.release()
    dsbp.release()
    probs.release()
    pCD.release()
    pBC.release()
    wfcp.release()
    xcp.release()
    stp2.release()
    pEF.release()
    pEG.release()
    ps_av.release()
    ps.release()
    dram.release()
    consts.release()



_CACHED = None


def _get_program():
    global _CACHED
    if _CACHED is None:
        _CACHED = build_program()
    return _CACHED


def _prep_inputs(inputs):
    """Fold LN params into weights and build the 8 per-core input maps."""
    x = np.asarray(inputs["x"], np.float32)
    ln1_w = np.asarray(inputs["ln1_w"], np.float32)
    ln1_b = np.asarray(inputs["ln1_b"], np.float32)
    w_attn = np.asarray(inputs["w_attn"], np.float32)
    b_attn = np.asarray(inputs["b_attn"], np.float32)
    w_proj = np.asarray(inputs["w_proj"], np.float32)
    b_proj = np.asarray(inputs["b_proj"], np.float32)
    ln2_w = np.asarray(inputs["ln2_w"], np.float32)
    ln2_b = np.asarray(inputs["ln2_b"], np.float32)
    w_fc = np.asarray(inputs["w_fc"], np.float32)
    b_fc = np.asarray(inputs["b_fc"], np.float32)
    w_mp = np.asarray(inputs["w_mlp_proj"], np.float32)
    b_mp = np.asarray(inputs["b_mlp_proj"], np.float32)

    Wa = ln1_w[:, None] * w_attn                      # [C, 3C]
    Ba = b_attn + ln1_b @ w_attn                      # [3C]
    s = 1.0 / np.sqrt(D)
    Wq = Wa[:, 0:C] * s
    Bq = Ba[0:C] * s
    Wk = Wa[:, C:2 * C]
    Bk = Ba[C:2 * C]
    Wv = Wa[:, 2 * C:3 * C]
    Bv = Ba[2 * C:3 * C]
    bproj_eff = (b_proj + Bv @ w_proj).astype(np.float32)

    Wfc = (ln2_w[:, None] * w_fc).astype(np.float32)
    Bfc = (b_fc + ln2_b @ w_fc).astype(np.float32)

    ident = np.eye(128, dtype=np.float16)
    gm = np.where(np.arange(128)[:, None] < np.arange(128)[None, :],
                  np.float16(-30.0), np.float16(0.0))

    in_maps = []
    for c in range(N_CORES):
        g, p = divmod(c, TP)
        hs = slice(HPC * D * p, HPC * D * (p + 1))    # 256 cols/rows per core
        wqk = np.ascontiguousarray(
            np.concatenate([Wq[:, hs], Wk[:, hs]], axis=1), np.float16)
        bqk = np.ascontiguousarray(
            np.concatenate([Bq[hs], Bk[hs]]), np.float32)
        xres = np.concatenate(
            [x[g][512 * j + 128 * p:512 * j + 128 * p + 128]
             for j in range(QC)], axis=0) + bproj_eff[None, :]
        in_maps.append({
            "x": np.ascontiguousarray(x[g]).astype(np.float16),
            "xres": np.ascontiguousarray(xres),
            "wqk": wqk,
            "bqk": bqk,
            "wv": np.ascontiguousarray(Wv[:, hs]).astype(np.float16),
            "wproj": np.ascontiguousarray(w_proj[hs, :]).astype(np.float16),
            "wfc": Wfc.astype(np.float16),
            "bfc": Bfc,
            "wmp": w_mp.astype(np.float16),
            "bmp": b_mp,
            "ident": ident,
            "gm": gm,
        })
    return in_maps


def _gather(results):
    out = np.empty((B, T, C), np.float32)
    for c in range(N_CORES):
        g, p = divmod(c, TP)
        for j in range(QC):
            out[g, 512 * j + 128 * p:512 * j + 128 * p + 128, :] = \
                results[c]["out"][128 * j:128 * (j + 1)]
    return out


def kernel(**inputs) -> np.ndarray:
    nc = _get_program()
    in_maps = _prep_inputs(inputs)
    res = run_bass_kernel_spmd(nc, in_maps, list(range(N_CORES)))
    return _gather(res.results)


if __name__ == "__main__":
    print("building program...")
    _get_program()
    print("built ok")


# revision 28
# speedup vs baseline: 1.3499x; 1.0509x over previous
"""Trainium2 Bass kernel for a dense transformer block (B=2, T=2048, C=1024, H=16).

Sharding: DP2 (batch -> core groups {0-3},{4-7}) x TP4 within a group:
  - attention: Megatron head-parallel (4 heads/core), row-parallel out-proj,
    pipelined ReduceScatter(add) over the group (one RS per 512-row block,
    issued immediately after that query-chunk's attention).
  - MLP: sequence-parallel (each core computes its 512 rows with the FULL
    fc / proj weights). No other collective.

Row ownership: core at group position p owns rows {512j+128p .. 512j+128p+128}
for j in 0..3 (one 128-row strip per pipelined ReduceScatter).

Device layout notes:
  - Activations feeding matmuls are kept transposed [features, tokens]
    ("^T layout") so every matmul contracts over the partition dim.
  - LN affine params are folded into the following weights on the host;
    q-scale (1/sqrt(D)) folded into W_q/b_q; v-bias folded into b_proj.
  - Softmax: scores^T[k,q] tiles; exp on ScalarE (no max subtraction:
    scores are ~N(0,1), safe); denominator via ones-column appended to V
    (row 64 of the PV matmul output); normalization applied to y^T with a
    DRAM-bounced partition-broadcast of 1/denom.
  - Matmul operands are fp16 (full PE rate, fast weight load); all
    accumulation, softmax statistics, residuals and LN are fp32.
  - x arrives fp16 (host-cast) to cut first-tile DMA latency; the fp32
    residual base (xres) is a separate input.
  - Scheduling: proj+RS issued per chunk right after its attention; MLP
    weights streamed once (strip-3 fc wave reuses the 2 resident groups);
    single wmp stream covers all 4 strips.
"""

import os
import sys

import numpy as np

for _p in ("/opt/trn_rl_repo", "/root/.axon_site/_ro/trn_rl_repo"):
    if os.path.isdir(_p) and _p not in sys.path:
        sys.path.insert(0, _p)

import concourse.bass as bass
import concourse.tile as tile
from concourse import bacc, mybir
from concourse.bass_utils import run_bass_kernel_spmd

B, T, C, H = 2, 2048, 1024, 16
D = C // H  # 64
EPS = 1e-5
N_CORES = 8
TP = 4            # tensor-parallel group size
HPC = 4           # heads per core
ROWS = T // TP    # 512 token rows owned per core
F32 = mybir.dt.float32
F16 = mybir.dt.float16  # matmul operand dtype

TT = T // 128     # 16 token tiles
CB = C // 128     # 8 channel blocks
QC = T // 512     # 4 query chunks / row blocks
RG = [[0, 1, 2, 3], [4, 5, 6, 7]]

GELU_NAME = "Gelu_apprx_tanh"  # sim_check overrides (sim lacks Gelu)


def _bc(ap, p):
    """Broadcast a DRAM AP across p partitions (prepend stride-0 dim)."""
    return bass.AP(tensor=ap.tensor, offset=ap.offset, ap=[[0, p], *ap.ap])


def build_program():
    nc = bacc.Bacc(
        "TRN2", target_bir_lowering=False, debug=False, num_devices=N_CORES
    )

    # ---- I/O ----
    x_d = nc.dram_tensor("x", [T, C], F16, kind="ExternalInput").ap()
    wqk_d = nc.dram_tensor("wqk", [C, 512], F16, kind="ExternalInput").ap()
    bqk_d = nc.dram_tensor("bqk", [512], F32, kind="ExternalInput").ap()
    wv_d = nc.dram_tensor("wv", [C, 256], F16, kind="ExternalInput").ap()
    wproj_d = nc.dram_tensor("wproj", [256, C], F16, kind="ExternalInput").ap()
    wfc_d = nc.dram_tensor("wfc", [C, 4 * C], F16, kind="ExternalInput").ap()
    bfc_d = nc.dram_tensor("bfc", [4 * C], F32, kind="ExternalInput").ap()
    wmp_d = nc.dram_tensor("wmp", [4 * C, C], F16, kind="ExternalInput").ap()
    bmp_d = nc.dram_tensor("bmp", [C], F32, kind="ExternalInput").ap()
    ident_d = nc.dram_tensor("ident", [128, 128], F16, kind="ExternalInput").ap()
    gm_d = nc.dram_tensor("gm", [128, 128], F16, kind="ExternalInput").ap()
    xres_d = nc.dram_tensor("xres", [ROWS, C], F32, kind="ExternalInput").ap()
    out_d = nc.dram_tensor("out", [ROWS, C], F32, kind="ExternalOutput").ap()

    with tile.TileContext(nc) as tc:
        _body(nc, tc, locals())
    nc.compile()
    return nc


def _body(nc, tc, io):
    x_d = io["x_d"]; wqk_d = io["wqk_d"]; bqk_d = io["bqk_d"]; wv_d = io["wv_d"]
    wproj_d = io["wproj_d"]; wfc_d = io["wfc_d"]
    bfc_d = io["bfc_d"]; wmp_d = io["wmp_d"]; bmp_d = io["bmp_d"]
    ident_d = io["ident_d"]; gm_d = io["gm_d"]; xres_d = io["xres_d"]
    out_d = io["out_d"]

    AF = mybir.ActivationFunctionType
    OP = mybir.AluOpType

    consts = tc.alloc_tile_pool(name="consts", bufs=1)
    dram = tc.alloc_tile_pool(name="dram", bufs=1, space="DRAM")
    ps = tc.alloc_tile_pool(name="ps", bufs=6, space="PSUM")
    ps_av = tc.alloc_tile_pool(name="ps_av", bufs=2, space="PSUM")

    # ---------- constants (DMAs issued lazily below; tiles just declared) ----
    ident = consts.tile([128, 128], F16)
    gm = consts.tile([128, 128], F16)  # -30 above the causal diagonal
    epsb = consts.tile([128, 1], F32)
    bqk_sb = consts.tile([128, 4], F32)
    bfc_sb = consts.tile([128, 32], F32)
    bmp_bc = consts.tile([128, C], F32)
    ones_c = consts.tile([128, HPC, 1], F16)
    ones_b = consts.tile([128, 64], F16)

    # DRAM scratch (fp16 collective payload)
    attn_part = dram.tile([T, C], F16)
    rs_out = [dram.tile([128, C], F16, tag=f"rs{j}", name=f"rs{j}")
              for j in range(QC)]
    dnrm = [dram.tile([HPC, 512], F32, tag=f"dn{j}", name=f"dn{j}")
            for j in range(QC)]

    # ======== Pools (alloc order must honor LIFO release points) ========
    pEG = tc.alloc_tile_pool(name="pEG", bufs=1)   # x_mid (residual base)
    pEF = tc.alloc_tile_pool(name="pEF", bufs=1)   # h_ln^T
    stp2 = tc.alloc_tile_pool(name="stp2", bufs=4)
    xcp = tc.alloc_tile_pool(name="xcp", bufs=2)
    wfcp = tc.alloc_tile_pool(name="wfcp", bufs=16)
    pBC = tc.alloc_tile_pool(name="pBC", bufs=1)   # Q^T/K^T + V natural
    pCD = tc.alloc_tile_pool(name="pCD", bufs=1)   # y^T + w_proj
    probs = tc.alloc_tile_pool(name="probs", bufs=8)
    dsbp = tc.alloc_tile_pool(name="dsbp", bufs=4)
    ystg = tc.alloc_tile_pool(name="ystg", bufs=2)
    prst = tc.alloc_tile_pool(name="prst", bufs=3)
    pAB = tc.alloc_tile_pool(name="pAB", bufs=1)   # x_ln^T + qkv weights
    xpool = tc.alloc_tile_pool(name="xpool", bufs=3)
    stp = tc.alloc_tile_pool(name="stp", bufs=4)

    xlnT = pAB.tile([128, CB, T], F16, name="xlnT")
    wqk_sb = [pAB.tile([128, 512], F16, tag=f"wqk{i}", name=f"wqk{i}")
              for i in range(CB)]
    wv_sb = [pAB.tile([128, 256], F16, tag=f"wv{i}", name=f"wv{i}")
             for i in range(CB)]
    # Q^T per head, zero-padded to 128 rows (head's 64-row band at its
    # position in the K-pair tile; the other band is zero). Full-partition
    # streaming keeps the PE_HAM activity monitor at full clock during
    # scores (64-contract matmuls otherwise read as "idle" -> K=4/8).
    qz = [pBC.tile([128, T], F16, tag=f"qz{i}", name=f"qz{i}")
          for i in range(4)]
    kT = [pBC.tile([128, T], F16, tag=f"kT{i}", name=f"kT{i}")
          for i in range(2)]  # K^T, 2 heads stacked per tile
    vnat = [pBC.tile([128, 260], F16, tag=f"vnat{i}", name=f"vnat{i}")
            for i in range(TT)]  # per head: 64 V cols + ones col (65 each)
    yT = [pCD.tile([128, T], F16, tag=f"yT{i}", name=f"yT{i}")
          for i in range(2)]  # y^T, 2 heads per tile
    wproj_sb = [pCD.tile([128, C], F16, tag=f"wp{i}", name=f"wp{i}")
                for i in range(2)]
    x_mid = [pEG.tile([128, C], F32, tag=f"xmid{i}", name=f"xmid{i}")
             for i in range(QC)]
    hlnT = pEF.tile([128, CB, ROWS], F16, name="hlnT")

    def load_x_chunk(tcn):
        """Prefetch the 4 x tiles of a token chunk (fp16, 256KB each)."""
        xts = []
        for tt in range(4 * tcn, 4 * tcn + 4):
            xt = xpool.tile([128, C], F16, tag="xt", bufs=4)
            nc.sync.dma_start(out=xt, in_=x_d[tt * 128:(tt + 1) * 128, :])
            xts.append(xt)
        return xts

    def feed(tcn, xts):
        """LN1, transpose, V-natural, and qkv^T for chunk (x pre-fetched)."""
        for i4, tt in enumerate(range(4 * tcn, 4 * tcn + 4)):
            xt = xts[i4]
            st = stp.tile([128, 2, 6], F32, tag="st")
            xr = xt.rearrange("p (g f) -> p g f", g=2)
            nc.vector.bn_stats(out=st[:, 0, :], in_=xr[:, 0, :])
            nc.vector.bn_stats(out=st[:, 1, :], in_=xr[:, 1, :])
            mv = stp.tile([128, 2], F32, tag="mv")
            nc.vector.bn_aggr(out=mv, in_=st)
            rstd = stp.tile([128, 1], F32, tag="rstd")
            nc.scalar.activation(out=rstd, in_=mv[:, 1:2], func=AF.Sqrt,
                                 bias=epsb, scale=1.0)
            nc.vector.reciprocal(out=rstd, in_=rstd)
            xc = xpool.tile([128, C], F16, tag="xc", bufs=3)
            nc.vector.tensor_scalar(out=xc, in0=xt, scalar1=mv[:, 0:1],
                                    scalar2=rstd, op0=OP.subtract,
                                    op1=OP.mult)
            for cq in range(2):  # two psum banks of 4 transposes each
                pt = ps.tile([128, 512], F16, tag="mm", name="pt")
                for i in range(4):
                    cb = cq * 4 + i
                    nc.tensor.matmul(
                        pt[:, 128 * i:128 * (i + 1)],
                        xc[:, cb * 128:(cb + 1) * 128], ident,
                        is_transpose=True, start=(i == 0), stop=(i == 3))
                nc.vector.tensor_copy(
                    out=xlnT[:, cq * 4:cq * 4 + 4, tt * 128:(tt + 1) * 128],
                    in_=pt.rearrange("p (i f) -> p i f", f=128))
            # V natural for this token tile
            pv = ps.tile([128, 256], F32, tag="mm", name="pv")
            for k in range(CB):
                nc.tensor.matmul(
                    pv, xlnT[:, k, tt * 128:(tt + 1) * 128],
                    wv_sb[k], start=(k == 0), stop=(k == CB - 1))
            nc.vector.tensor_copy(
                out=vnat[tt].rearrange("p (h x) -> p h x", x=65)[:, :, 64:65],
                in_=ones_c)
            nc.vector.tensor_copy(
                out=vnat[tt].rearrange("p (h x) -> p h x", x=65)[:, :, 0:64],
                in_=pv.rearrange("p (h x) -> p h x", x=64))
        # Q^T/K^T columns for this token chunk
        cs = slice(tcn * 512, (tcn + 1) * 512)
        for mt in range(4):
            pq = ps.tile([128, 512], F32, tag="mm", name="pq")
            for k in range(CB):
                nc.tensor.matmul(
                    pq, wqk_sb[k][:, mt * 128:(mt + 1) * 128],
                    xlnT[:, k, tcn * 512:(tcn + 1) * 512],
                    start=(k == 0), stop=(k == CB - 1))
            if mt < 2:  # Q: split the head pair into the padded per-head tiles
                nc.vector.tensor_scalar_add(
                    out=qz[2 * mt][0:64, cs], in0=pq[0:64, :],
                    scalar1=bqk_sb[0:64, mt:mt + 1])
                nc.vector.tensor_scalar_add(
                    out=qz[2 * mt + 1][64:128, cs], in0=pq[64:128, :],
                    scalar1=bqk_sb[64:128, mt:mt + 1])
            else:
                nc.vector.tensor_scalar_add(
                    out=kT[mt - 2][:, cs], in0=pq,
                    scalar1=bqk_sb[:, mt:mt + 1])

    GRP = 4  # scores emitted in shape-uniform groups; PV trails one group

    def attention(qc, carry, heads):
        """carry: list of deferred (off, ysl, d16) normalizations."""
        for h in heads:
            off = 64 * (h % 2)
            qh = qz[h]   # 128 rows: head band + zeros
            kh = kT[h // 2]  # other head's rows hit Q's zero band
            nkb = 4 * qc + 4
            py = ps_av.tile([128, 512], F32, tag="py", name="py")
            pend = []
            for g0 in range(0, nkb, GRP):
                prs = []
                for kb in range(g0, min(g0 + GRP, nkb)):
                    j = kb - 4 * qc
                    lo = max(j, 0) * 128  # fully-masked columns skipped
                    pss = ps.tile([128, 512], F32, tag="mm", name="pss")
                    nc.tensor.matmul(
                        pss[:, lo:512], kh[:, kb * 128:(kb + 1) * 128],
                        qh[:, qc * 512 + lo:(qc + 1) * 512],
                        start=True, stop=(j < 0))
                    if j >= 0:  # causal diagonal: accumulate -30 above it
                        nc.tensor.matmul(
                            pss[:, lo:lo + 128], gm, ident,
                            start=False, stop=True, skip_group_check=True)
                    pr = probs.tile([128, 512], F16, tag="pr")
                    nc.scalar.activation(out=pr[:, lo:512],
                                         in_=pss[:, lo:512], func=AF.Exp)
                    prs.append((kb, lo, pr))
                if g0 == 0 and carry:
                    # one deferred y^T normalization per head (spacing)
                    _flush_one(carry.pop(0))
                for pkb, plo, ppr in pend:  # PV for the previous group
                    nc.tensor.matmul(
                        py[0:65, plo:512], vnat[pkb][:, h * 65:h * 65 + 65],
                        ppr[:, plo:512], start=(pkb == 0),
                        stop=(pkb == nkb - 1))
                pend = prs
            for pkb, plo, ppr in pend:
                nc.tensor.matmul(
                    py[0:65, plo:512], vnat[pkb][:, h * 65:h * 65 + 65],
                    ppr[:, plo:512], start=(pkb == 0), stop=(pkb == nkb - 1))
            # 1/denominator -> DRAM-bounced partition broadcast (deferred).
            # Staging copy on ScalarE; reciprocal reads SBUF (single-src
            # perf mode), keeping the PSUM-read penalty off VectorE.
            dsb = dsbp.tile([65, 512], F32, tag="dsb", bufs=2)
            nc.scalar.copy(out=dsb[64:65, :], in_=py[64:65, :])
            nc.vector.reciprocal(out=dsb[64:65, :], in_=dsb[64:65, :])
            nc.sync.dma_start(out=dnrm[qc][h, :], in_=dsb[64:65, :])
            rbc = dsbp.tile([64, 512], F32, tag="rbc", bufs=4)
            nc.sync.dma_start(out=rbc, in_=_bc(dnrm[qc][h, :], 64))
            ysl = yT[h // 2][off:off + 64, qc * 512:(qc + 1) * 512]
            carry.append((h, ysl, rbc, py))
        return carry

    def _flush_one(ent):
        # y^T = py * (1/den): single TT op reading the PV PSUM bank
        h0, ysl0, rbc0, py0 = ent
        if h0 % 2 == 0:
            nc.vector.tensor_mul(ysl0, py0[0:64, :], rbc0)
        else:
            yst = ystg.tile([64, 512], F16, tag="yst")
            nc.vector.tensor_mul(yst, py0[0:64, :], rbc0)
            nc.sync.dma_start(out=ysl0, in_=yst)

    def flush_norm(carry):
        for ent in carry:
            _flush_one(ent)
        carry.clear()

    def proj_rs(qc):
        for tt in range(4 * qc, 4 * qc + 4):
            for cc in range(2):
                pp = ps.tile([128, 512], F32, tag="mm", name="pp")
                for k in range(2):
                    nc.tensor.matmul(
                        pp, yT[k][:, tt * 128:(tt + 1) * 128],
                        wproj_sb[k][:, cc * 512:(cc + 1) * 512],
                        start=(k == 0), stop=(k == 1))
                pst = prst.tile([128, 512], F16, tag="pst")
                nc.vector.tensor_copy(out=pst, in_=pp)
                nc.sync.dma_start(
                    out=attn_part[tt * 128:(tt + 1) * 128,
                                  cc * 512:(cc + 1) * 512],
                    in_=pst)
        nc.gpsimd.collective_compute(
            "ReduceScatter", mybir.AluOpType.add, replica_groups=RG,
            ins=[attn_part[qc * 512:(qc + 1) * 512, :].opt()],
            outs=[rs_out[qc].opt()])

    def resid_ln2(qc, t_rst, t_rest):
        # residual + LN2 + h_ln^T for the owned 128-row strip.  The rst
        # load is hinted at the RS *trigger* time (so it lands on the
        # gpsimd queue before the next RS trigger); the compute chain is
        # hinted at the RS *completion* time (head-of-line avoidance on
        # the in-order vector queue).
        with tc.tile_wait_until(t_rst):
            xo = xcp.tile([128, C], F32, tag="xo")
            nc.sync.dma_start(out=xo, in_=xres_d[qc * 128:(qc + 1) * 128, :])
            rst = xcp.tile([128, C], F16, tag="rst")
            nc.gpsimd.dma_start(out=rst, in_=rs_out[qc])
        with tc.tile_wait_until(t_rest):
            _resid_ln2(qc, rst, xo)

    def _resid_ln2(qc, rst, xo):
        nc.vector.tensor_add(x_mid[qc], rst, xo)
        st = stp2.tile([128, 2, 6], F32, tag="st2")
        xr = x_mid[qc].rearrange("p (g f) -> p g f", g=2)
        nc.vector.bn_stats(out=st[:, 0, :], in_=xr[:, 0, :])
        nc.vector.bn_stats(out=st[:, 1, :], in_=xr[:, 1, :])
        mv = stp2.tile([128, 2], F32, tag="mv2")
        nc.vector.bn_aggr(out=mv, in_=st)
        rstd = stp2.tile([128, 1], F32, tag="rstd2")
        nc.scalar.activation(out=rstd, in_=mv[:, 1:2], func=AF.Sqrt,
                             bias=epsb, scale=1.0)
        nc.vector.reciprocal(out=rstd, in_=rstd)
        xc = xcp.tile([128, C], F16, tag="xc2")
        nc.vector.tensor_scalar(out=xc, in0=x_mid[qc], scalar1=mv[:, 0:1],
                                scalar2=rstd, op0=OP.subtract, op1=OP.mult)
        for cq in range(2):
            pt = ps.tile([128, 512], F16, tag="mm", name="pt2")
            for i in range(4):
                cb = cq * 4 + i
                nc.tensor.matmul(
                    pt[:, 128 * i:128 * (i + 1)],
                    xc[:, cb * 128:(cb + 1) * 128], ident,
                    is_transpose=True, start=(i == 0), stop=(i == 3))
            nc.vector.tensor_copy(
                out=hlnT[:, cq * 4:cq * 4 + 4, qc * 128:(qc + 1) * 128],
                in_=pt.rearrange("p (i f) -> p i f", f=128))

    def load_wfc_group(mg, tag2, t_load):
        wg = []
        with tc.tile_wait_until(t_load):
            for k in range(CB):
                w = wfcp.tile([128, 1024], F16, tag="wfc",
                              name=f"wfc{tag2}_{mg}_{k}")
                nc.scalar.dma_start(
                    out=w, in_=wfc_d[k * 128:(k + 1) * 128,
                                     mg * 1024:(mg + 1) * 1024])
                wg.append(w)
        return wg

    def fc_mg(mg, t0, t1, wg, h2gT):
        # h2^T = gelu(wfc^T @ h_ln^T + b_fc), one mg weight group, rows [t0,t1)
        n0, n1 = t0 * 128, t1 * 128
        for mt in range(8):
            m = mg * 8 + mt
            pf = ps.tile([128, 512], F32, tag="mm", name="pf")
            for k in range(CB):
                nc.tensor.matmul(
                    pf[:, 0:n1 - n0], wg[k][:, mt * 128:(mt + 1) * 128],
                    hlnT[:, k, n0:n1], start=(k == 0),
                    stop=(k == CB - 1))
            nc.scalar.activation(
                out=h2gT[:, m, n0:n1], in_=pf[:, 0:n1 - n0],
                func=getattr(AF, GELU_NAME),
                bias=bfc_sb[:, m:m + 1], scale=1.0)

    def g_pass(tlist, h2gT):
        # out rows = h2g^T.T @ wmp + x_mid for the given strips
        for cc in range(2):
            pg = {t: ps.tile([128, 512], F32, tag="mm", name=f"pg{cc}_{t}")
                  for t in tlist}
            for k in range(32):
                wm = wmpp.tile([128, 512], F16, tag="wmp")
                nc.sync.dma_start(
                    out=wm, in_=wmp_d[k * 128:(k + 1) * 128,
                                      cc * 512:(cc + 1) * 512])
                for t in tlist:
                    nc.tensor.matmul(
                        pg[t], h2gT[:, k, t * 128:(t + 1) * 128],
                        wm, start=(k == 0), stop=(k == 31))
            for t in tlist:
                ot = outp.tile([128, 512], F32, tag="ot")
                nc.vector.tensor_add(ot, pg[t],
                                     x_mid[t][:, cc * 512:(cc + 1) * 512])
                nc.vector.tensor_add(ot, ot,
                                     bmp_bc[:, cc * 512:(cc + 1) * 512])
                nc.sync.dma_start(
                    out=out_d[t * 128:(t + 1) * 128,
                              cc * 512:(cc + 1) * 512],
                    in_=ot)

    # ====== unified software pipeline over token/query chunks ======
    # DMA issue order = priority: x chunk 0 first, then ident (transposes),
    # wv/wqk (feed 0), trim (attn 0); heavier / later-needed loads follow.
    xts0 = load_x_chunk(0)
    nc.sync.dma_start(out=ident, in_=ident_d)
    nc.vector.memset(epsb, EPS)
    nc.vector.memset(ones_c, 1.0)
    nc.vector.memset(ones_b, 1.0)
    for h in range(4):  # zero the unused 64-row band of each padded Q^T
        zoff = 0 if h % 2 else 64
        nc.vector.memset(qz[h][zoff:zoff + 64, :], 0.0)
    for k in range(CB):
        nc.sync.dma_start(out=wv_sb[k], in_=wv_d[k * 128:(k + 1) * 128, :])
    for k in range(CB):
        nc.sync.dma_start(out=wqk_sb[k], in_=wqk_d[k * 128:(k + 1) * 128, :])
    nc.sync.dma_start(out=bqk_sb, in_=bqk_d.rearrange("(m p) -> p m", p=128))
    nc.sync.dma_start(out=gm, in_=gm_d)
    xts1 = load_x_chunk(1)
    for k in range(2):
        nc.sync.dma_start(out=wproj_sb[k],
                          in_=wproj_d[k * 128:(k + 1) * 128, :])

    carry = []
    feed(0, xts0)
    attention(0, carry, range(HPC))
    xts2 = load_x_chunk(2)
    # deferred const loads (needed from resid_ln2 / fc onwards)
    nc.sync.dma_start(out=bmp_bc, in_=_bc(bmp_d, 128))
    nc.sync.dma_start(out=bfc_sb, in_=bfc_d.rearrange("(m p) -> p m", p=128))
    feed(1, xts1)
    flush_norm(carry)
    proj_rs(0)
    attention(1, carry, range(HPC))
    xts3 = load_x_chunk(3)
    feed(2, xts2)
    flush_norm(carry)
    proj_rs(1)
    resid_ln2(0, 0.100, 0.130)
    attention(2, carry, range(HPC))
    feed(3, xts3)
    flush_norm(carry)
    proj_rs(2)
    resid_ln2(1, 0.158, 0.180)
    attention(3, carry, range(HPC))
    flush_norm(carry)
    resid_ln2(2, 0.232, 0.252)
    proj_rs(3)
    stp.release()
    xpool.release()
    pAB.release()
    # MLP pools open only after the attention pools close (SBUF budget)
    pFG = tc.alloc_tile_pool(name="pFG", bufs=1)   # gelu(h2)^T
    wmpp = tc.alloc_tile_pool(name="wmpp", bufs=8)
    outp = tc.alloc_tile_pool(name="outp", bufs=3)
    h2gT = pFG.tile([128, 32, ROWS], F16, name="h2gT")
    # fc wave 1: strips 0-2; wfc groups streamed once (bufs=16 -> the last
    # two groups stay resident for the strip-3 wave)
    wgs = {mg: load_wfc_group(mg, "a", 0.200 + 0.012 * mg)
           for mg in range(4)}
    for mg in range(4):
        with tc.tile_wait_until(0.280 + 0.004 * mg):
            fc_mg(mg, 0, 3, wgs[mg], h2gT)
    resid_ln2(3, 0.302, 0.333)
    # fc wave 2: strip 3. mg3/mg2 still resident; mg1/mg0 re-streamed.
    with tc.tile_wait_until(0.350):
        fc_mg(3, 3, 4, wgs[3], h2gT)
        fc_mg(2, 3, 4, wgs[2], h2gT)
        fc_mg(1, 3, 4, load_wfc_group(1, "b", 0.310), h2gT)
        fc_mg(0, 3, 4, load_wfc_group(0, "b", 0.316), h2gT)
    # single wmp stream over all 4 strips
    with tc.tile_wait_until(0.370):
        g_pass([0, 1, 2, 3], h2gT)

    outp.release()
    wmpp.release()
    pFG.release()
    prst.release()
    ystg.release()
    dsbp.release()
    probs.release()
    pCD.release()
    pBC.release()
    wfcp.release()
    xcp.release()
    stp2.release()
    pEF.release()
    pEG.release()
    ps_av.release()
    ps.release()
    dram.release()
    consts.release()



_CACHED = None


def _get_program():
    global _CACHED
    if _CACHED is None:
        _CACHED = build_program()
    return _CACHED


def _prep_inputs(inputs):
    """Fold LN params into weights and build the 8 per-core input maps."""
    x = np.asarray(inputs["x"], np.float32)
    ln1_w = np.asarray(inputs["ln1_w"], np.float32)
    ln1_b = np.asarray(inputs["ln1_b"], np.float32)
    w_attn = np.asarray(inputs["w_attn"], np.float32)
    b_attn = np.asarray(inputs["b_attn"], np.float32)
    w_proj = np.asarray(inputs["w_proj"], np.float32)
    b_proj = np.asarray(inputs["b_proj"], np.float32)
    ln2_w = np.asarray(inputs["ln2_w"], np.float32)
    ln2_b = np.asarray(inputs["ln2_b"], np.float32)
    w_fc = np.asarray(inputs["w_fc"], np.float32)
    b_fc = np.asarray(inputs["b_fc"], np.float32)
    w_mp = np.asarray(inputs["w_mlp_proj"], np.float32)
    b_mp = np.asarray(inputs["b_mlp_proj"], np.float32)

    Wa = ln1_w[:, None] * w_attn                      # [C, 3C]
    Ba = b_attn + ln1_b @ w_attn                      # [3C]
    s = 1.0 / np.sqrt(D)
    Wq = Wa[:, 0:C] * s
    Bq = Ba[0:C] * s
    Wk = Wa[:, C:2 * C]
    Bk = Ba[C:2 * C]
    Wv = Wa[:, 2 * C:3 * C]
    Bv = Ba[2 * C:3 * C]
    bproj_eff = (b_proj + Bv @ w_proj).astype(np.float32)

    Wfc = (ln2_w[:, None] * w_fc).astype(np.float32)
    Bfc = (b_fc + ln2_b @ w_fc).astype(np.float32)

    ident = np.eye(128, dtype=np.float16)
    gm = np.where(np.arange(128)[:, None] < np.arange(128)[None, :],
                  np.float16(-30.0), np.float16(0.0))

    in_maps = []
    for c in range(N_CORES):
        g, p = divmod(c, TP)
        hs = slice(HPC * D * p, HPC * D * (p + 1))    # 256 cols/rows per core
        wqk = np.ascontiguousarray(
            np.concatenate([Wq[:, hs], Wk[:, hs]], axis=1), np.float16)
        bqk = np.ascontiguousarray(
            np.concatenate([Bq[hs], Bk[hs]]), np.float32)
        xres = np.concatenate(
            [x[g][512 * j + 128 * p:512 * j + 128 * p + 128]
             for j in range(QC)], axis=0) + bproj_eff[None, :]
        in_maps.append({
            "x": np.ascontiguousarray(x[g]).astype(np.float16),
            "xres": np.ascontiguousarray(xres),
            "wqk": wqk,
            "bqk": bqk,
            "wv": np.ascontiguousarray(Wv[:, hs]).astype(np.float16),
            "wproj": np.ascontiguousarray(w_proj[hs, :]).astype(np.float16),
            "wfc": Wfc.astype(np.float16),
            "bfc": Bfc,
            "wmp": w_mp.astype(np.float16),
            "bmp": b_mp,
            "ident": ident,
            "gm": gm,
        })
    return in_maps


def _gather(results):
    out = np.empty((B, T, C), np.float32)
    for c in range(N_CORES):
        g, p = divmod(c, TP)
        for j in range(QC):
            out[g, 512 * j + 128 * p:512 * j + 128 * p + 128, :] = \
                results[c]["out"][128 * j:128 * (j + 1)]
    return out


def kernel(**inputs) -> np.ndarray:
    nc = _get_program()
    in_maps = _prep_inputs(inputs)
    res = run_bass_kernel_spmd(nc, in_maps, list(range(N_CORES)))
    return _gather(res.results)


if __name__ == "__main__":
    print("building program...")
    _get_program()
    print("built ok")
